# revision 1
# baseline (speedup 1.0000x reference)
"""DownBlock kernel builder for trn2 (8-core SPMD, Bass/Tile).

See kernel.py for the pipeline description.
"""
import sys

sys.path.insert(0, "/opt/trn_rl_repo")

import contextlib

import numpy as np

import concourse.bass as bass
import concourse.bacc as bacc
import concourse.mybir as mybir
import concourse.tile as tile
from concourse.masks import make_identity

F32 = mybir.dt.float32
BF16 = mybir.dt.bfloat16
I32 = mybir.dt.int32
AF = mybir.ActivationFunctionType
OP = mybir.AluOpType
EPS = 1e-5
ALPHA = 0.2
P = 128


def bcast(ap_obj, pos, count):
    """Insert a zero-stride dim of `count` at free-dim position `pos` (0 = first
    free dim)."""
    new = bass.AP(ap_obj.tensor, ap_obj.offset,
                  [list(d) for d in ap_obj.ap])
    new.ap.insert(pos + 1, [0, count])
    return new


class Cfg:
    def __init__(self, B=16, C1=32, C2=64, K=7, VF=163842, VC=40962, n_cores=8,
                 use_lrelu=True, stop_after=None):
        self.B, self.C1, self.C2, self.K = B, C1, C2, K
        self.VF, self.VC, self.n_cores = VF, VC, n_cores
        self.R1 = B * C1
        self.R2 = B * C2
        tiles_total = -(-VC // P)
        self.TPC = -(-tiles_total // n_cores)
        self.VC_pad = self.TPC * n_cores * P
        self.Vs = self.TPC * P
        self.use_lrelu = use_lrelu
        self.stop_after = stop_after
        assert C1 == 32 and C2 % 32 == 0


def host_prep(cfg, x, w1, w2, g1, beta1, g2, beta2, conv_neigh, down_neigh):
    B, C1, C2, K = cfg.B, cfg.C1, cfg.C2, cfg.K
    VF, VC = cfg.VF, cfg.VC
    x = np.asarray(x, np.float32)
    conv_neigh = np.asarray(conv_neigh).astype(np.int32)
    down_neigh = np.asarray(down_neigh).astype(np.int32)
    w1 = np.asarray(w1, np.float32)
    w2 = np.asarray(w2, np.float32)

    xT = np.ascontiguousarray(x.transpose(2, 0, 1).reshape(VF, B * C1))

    # W1rep[vb*32+c, k*C2+o] = w1[o, c*K+k] / K
    w1r = w1.reshape(C2, C1, K)             # [o, c, k]
    W1blk = (w1r.transpose(1, 2, 0) / K)    # [c, k, o]
    W1rep = np.tile(W1blk.reshape(C1, K * C2), (4, 1)).astype(np.float32)
    # W2rep[vb*32+om, (k*n_ob+ob)*C2+o2] = w2[o2, (ob*32+om)*K+k]
    n_ob = C2 // 32
    w2r = w2.reshape(C2, C2, K)             # [o2, o, k] with o = ob*32+om
    W2blk = w2r.reshape(C2, n_ob, 32, K).transpose(2, 3, 1, 0)  # [om, k, ob, o2]
    W2blk = W2blk.transpose(0, 1, 2, 3).reshape(32, K * n_ob * C2)
    W2rep = np.tile(W2blk, (4, 1)).astype(np.float32)

    gvec = np.stack([np.asarray(g1, np.float32), np.asarray(beta1, np.float32),
                     np.asarray(g2, np.float32), np.asarray(beta2, np.float32)],
                    axis=1)

    per_core = []
    for ci in range(cfg.n_cores):
        sh0 = ci * cfg.Vs
        vid = sh0 + np.arange(cfg.Vs)
        valid = vid < VC
        vv = np.where(valid, vid, 0)
        pool_idx = down_neigh[vv].copy()
        pool_idx[~valid] = 0
        ci_raw = conv_neigh[vv]
        conv_idx = (ci_raw // cfg.Vs) * (cfg.Vs + 1) + ci_raw % cfg.Vs
        conv_idx = conv_idx.astype(np.int32)
        conv_idx[~valid] = cfg.Vs  # core-0 zero row
        TPC = cfg.TPC
        TH0 = (TPC + 1) // 2
        nidx = K * P
        xt_halves, packed_halves = [], []
        for h, (t0, t1) in enumerate([(0, TH0), (TH0, TPC)]):
            sl = pool_idx[t0 * P:t1 * P, :]
            uniq, inv = np.unique(sl, return_inverse=True)
            assert len(uniq) < 32700, f"pool half rows {len(uniq)} > int16"
            xt_halves.append(xT[uniq])
            pidx = inv.reshape(sl.shape).astype(np.int16)
            nt = t1 - t0
            packed = np.zeros((nt * P, nidx // 16), np.int16)
            for t in range(nt):
                lst = pidx[t * P:(t + 1) * P, :].T.reshape(nidx)
                blk = lst.reshape(nidx // 16, 16).T
                packed[t * P:(t + 1) * P, :] = np.tile(blk, (8, 1))
            packed_halves.append(packed)
        per_core.append(dict(
            xt0=xt_halves[0], xt1=xt_halves[1],
            pool_idx=np.concatenate(packed_halves, axis=0),
            conv_idx=conv_idx,
            w1rep=W1rep, w2rep=W2rep, gvec=gvec,
        ))
    mx = max(max(pc["xt0"].shape[0], pc["xt1"].shape[0]) for pc in per_core)
    for pc in per_core:
        for nm in ("xt0", "xt1"):
            n = pc[nm].shape[0]
            if n < mx:
                pc[nm] = np.concatenate(
                    [pc[nm], np.zeros((mx - n, B * C1), np.float32)], 0)
            pc[nm] = np.ascontiguousarray(pc[nm], dtype=np.float32)
    return mx, per_core


def build(cfg, xt_rows_max):
    B, C2, K = cfg.B, cfg.C2, cfg.K
    R1 = cfg.R1
    n_ob = C2 // 32
    nc = bacc.Bacc("TRN2", target_bir_lowering=False, debug=False,
                   num_devices=cfg.n_cores)
    xtt = (nc.dram_tensor("xt0", [xt_rows_max, R1], F32, kind="ExternalInput").ap(),
           nc.dram_tensor("xt1", [xt_rows_max, R1], F32, kind="ExternalInput").ap())
    pool_idx = nc.dram_tensor("pool_idx", [cfg.Vs, K * P // 16],
                          mybir.dt.int16, kind="ExternalInput").ap()
    conv_idx = nc.dram_tensor("conv_idx", [cfg.Vs, K], I32, kind="ExternalInput").ap()
    w1rep = nc.dram_tensor("w1rep", [128, K * C2], F32, kind="ExternalInput").ap()
    w2rep = nc.dram_tensor("w2rep", [128, K * n_ob * C2], F32,
                           kind="ExternalInput").ap()
    gvec = nc.dram_tensor("gvec", [C2, 4], F32, kind="ExternalInput").ap()
    out = nc.dram_tensor("out", [B, C2, cfg.Vs], F32, kind="ExternalOutput").ap()

    with tile.TileContext(nc) as tc:
        build_body(tc, cfg, xtt, pool_idx, conv_idx, w1rep, w2rep, gvec, out)
    nc.compile()
    return nc


def build_body(tc, cfg, xt, pool_idx, conv_idx, w1rep, w2rep, gvec, out):
    nc = tc.nc
    B, C1, C2, K = cfg.B, cfg.C1, cfg.C2, cfg.K
    R1, R2 = cfg.R1, cfg.R2
    TPC, Vs, VC_pad = cfg.TPC, cfg.Vs, cfg.VC_pad
    NC = cfg.n_cores
    n_ob = C2 // 32
    N_stat = float(B * cfg.VC)
    NVB = 4
    BCOL = B * 32   # psum free size per tile (b, vr)

    ctx = contextlib.ExitStack()
    with ctx:
        dram = ctx.enter_context(tc.tile_pool(name="dram", bufs=1, space="DRAM"))
        sb1 = ctx.enter_context(tc.tile_pool(name="sb1", bufs=1))
        sbg = ctx.enter_context(tc.tile_pool(name="sbg", bufs=2))
        psp = ctx.enter_context(tc.tile_pool(name="psp", bufs=2, space="PSUM"))

        xp_shard = dram.tile([Vs + 1, R1], F32, name="xp_shard")
        xp_full = dram.tile([(Vs + 1) * NC, R1], F32, name="xp_full",
                            addr_space="Shared")
        h_shard = dram.tile([Vs + 1, R2], BF16, name="h_shard")
        h_full = dram.tile([(Vs + 1) * NC, R2], BF16, name="h_full",
                           addr_space="Shared")
        ar_in = dram.tile([C2, 2], F32, name="ar_in")
        ar_out = dram.tile([C2, 2], F32, name="ar_out", addr_space="Shared")
        ar_in2 = dram.tile([C2, 2], F32, name="ar_in2")
        ar_out2 = dram.tile([C2, 2], F32, name="ar_out2", addr_space="Shared")

        w1_sb = sb1.tile([128, K * C2], F32)
        nc.sync.dma_start(w1_sb[:], w1rep[:])
        w2_sb32 = sb1.tile([128, K * n_ob * C2], F32)
        nc.sync.dma_start(w2_sb32[:], w2rep[:])
        w2_sb = sb1.tile([128, K * n_ob * C2], BF16)
        nc.vector.tensor_copy(w2_sb[:], w2_sb32[:])
        g_sb = sb1.tile([C2, 4], F32)
        nc.sync.dma_start(g_sb[:], gvec[:])
        ident = sb1.tile([128, 128], F32)
        make_identity(nc, ident[:])
        zero_sb = sb1.tile([128, R1], F32)
        nc.vector.memset(zero_sb[:], 0.0)
        zero_bf = sb1.tile([128, R2], BF16)
        nc.vector.memset(zero_bf[:], 0.0)
        alpha_sb = sb1.tile([128, 1], F32)
        nc.vector.memset(alpha_sb[:], ALPHA)
        cfg._alpha_sb = alpha_sb

        h_pre = dram.tile([Vs, R2], F32, name="h_pre")
        stats1 = sb1.tile([C2, TPC * NVB * 2], F32)
        stats2 = sb1.tile([C2, TPC * NVB * 2], F32)

        # =========== P1: pool ===========
        NIDX = K * P
        TH0 = (TPC + 1) // 2
        for t in range(TPC):
            idx_t = sbg.tile([P, NIDX // 16], mybir.dt.int16, tag="pidx", bufs=3)
            nc.sync.dma_start(idx_t[:], pool_idx[t * P:(t + 1) * P, :])
            gp = sbg.tile([P, K * R1], F32, tag="g1")
            nc.gpsimd.dma_gather(
                out_ap=gp[:].rearrange("p (n e) -> p n e", e=R1),
                in_ap=xt[0][:] if t < TH0 else xt[1][:], idxs_ap=idx_t[:],
                num_idxs=NIDX, num_idxs_reg=NIDX, elem_size=R1,
            )
            acc = sbg.tile([P, R1], F32, tag="poolacc", bufs=3)
            nc.vector.tensor_reduce(
                out=acc[:],
                in_=bass.AP(gp.tensor, gp[:].offset,
                            [list(gp[:].ap[0]), [1, R1], [R1, K]]),
                axis=mybir.AxisListType.X, op=OP.add)
            nc.sync.dma_start(xp_shard[t * P:(t + 1) * P, :], acc[:])

        nc.sync.dma_start(xp_shard[Vs:Vs + 1, :], zero_sb[0:1, 0:R1])
        if cfg.stop_after == "pool":
            fin = sbg.tile([P, R1], F32, tag="fin")
            nc.sync.dma_start(fin[:], xp_shard[0:P, :])
            nc.sync.dma_start(out[0, 0:64, 0:R1], fin[0:64, :])
            return
        nc.gpsimd.collective_compute(
            "AllGather", OP.bypass, replica_groups=[list(range(NC))],
            ins=[xp_shard[:].opt()], outs=[xp_full[:].opt()],
        )

        # =========== C1: conv1 ===========
        for t in range(TPC):
            idx_t = sbg.tile([P, K], I32, tag="cidx", bufs=3)
            nc.sync.dma_start(idx_t[:], conv_idx[t * P:(t + 1) * P, :])
            g1t = sbg.tile([P, K * R1], F32, tag="g1")
            for k in range(K):
                nc.gpsimd.indirect_dma_start(
                    out=g1t[:, k * R1:(k + 1) * R1], out_offset=None, in_=xp_full[:],
                    in_offset=bass.IndirectOffsetOnAxis(ap=idx_t[:, k:k + 1], axis=0),
                )
            t1 = sbg.tile([P, K * R1], F32, tag="t1")
            nc.vector.transpose(
                out=t1[:].rearrange("p (k b v) -> p k b v", k=K, b=B),
                in_=g1t[:].rearrange("p (k b c) -> p k b c", k=K, b=B),
            )
            psums = [psp.tile([C2, BCOL], F32, space="PSUM",
                              name=f"ps1_{t}_{vb}", tag=f"psA_{vb}")
                     for vb in range(NVB)]
            for k in range(K):
                for vb in range(NVB):
                    nc.tensor.matmul(
                        out=psums[vb][:, :],
                        lhsT=w1_sb[vb * 32:(vb + 1) * 32, k * C2:(k + 1) * C2],
                        rhs=t1[vb * 32:(vb + 1) * 32, k * R1:(k + 1) * R1],
                        start=(k == 0), stop=(k == K - 1),
                        tile_position=(vb * 32, 0),
                    )
            for vb in range(NVB):
                sc = t * NVB + vb
                dump = sbg.tile([C2, BCOL], F32, tag="dump", bufs=2)
                nc.scalar.activation(
                    out=dump[:], in_=psums[vb][:], func=AF.Identity,
                    accum_out=stats1[:, 2 * sc:2 * sc + 1])
                dump2 = sbg.tile([C2, BCOL], F32, tag="dump", bufs=2)
                nc.scalar.activation(
                    out=dump2[:], in_=psums[vb][:], func=AF.Square,
                    accum_out=stats1[:, 2 * sc + 1:2 * sc + 2])
                ht = sbg.tile([C2, BCOL], F32, tag="ht", bufs=4)
                nc.vector.transpose(
                    out=ht[:].rearrange("p (b v) -> p b v", b=B),
                    in_=psums[vb][:].rearrange("p (b v) -> p b v", b=B),
                )
                for ob in range(n_ob):
                    nc.sync.dma_start(
                        h_pre[t * P + vb * 32: t * P + (vb + 1) * 32,
                              ob * BCOL:(ob + 1) * BCOL],
                        ht[ob * 32:(ob + 1) * 32, :])

        # ---- BN1: stats AR, coeffs, apply, write h, AG ----
        _stats_reduce(nc, sb1, stats1, ar_in, ar_out, NC, "st1")
        glob1 = sb1.tile([C2, 2], F32)
        nc.sync.dma_start(glob1[:], ar_out[:])
        a1, c1 = _bn_coeffs(nc, sb1, glob1, g_sb[:, 0:1], g_sb[:, 1:2], N_stat, "bn1")
        a1m, c1m = _bn_rowmaps(nc, sb1, psp, ident, a1, c1, C2, B, n_ob, "b1m")

        for t in range(TPC):
            ha = sbg.tile([P, R2], F32, tag="happly", bufs=3)
            nc.sync.dma_start(ha[:], h_pre[t * P:(t + 1) * P, :])
            nc.vector.tensor_tensor(out=ha[:], in0=ha[:], in1=a1m[:, :],
                                    op=OP.mult)
            nc.vector.tensor_tensor(out=ha[:], in0=ha[:], in1=c1m[:, :],
                                    op=OP.add)
            _lrelu(nc, cfg, sbg, ha[:])
            nc.gpsimd.dma_start(h_shard[t * P:(t + 1) * P, :], ha[:])
        nc.sync.dma_start(h_shard[Vs:Vs + 1, :], zero_bf[0:1, :])
        if cfg.stop_after == "conv1":
            fin = sbg.tile([P, R2], BF16, tag="finb")
            nc.sync.dma_start(fin[:], h_shard[0:P, :])
            fin2 = sbg.tile([P, R2], F32, tag="finb2")
            nc.vector.tensor_copy(fin2[:], fin[:])
            nc.sync.dma_start(out[0, 0:64, 0:R2], fin2[0:64, :])
            return
        nc.gpsimd.collective_compute(
            "AllGather", OP.bypass, replica_groups=[list(range(NC))],
            ins=[h_shard[:].opt()], outs=[h_full[:].opt()],
        )

        # =========== C2: conv2 ===========
        GRP = 1
        stage = None
        for t in range(TPC):
            idx_t = sbg.tile([P, K], I32, tag="cidx2", bufs=3)
            nc.sync.dma_start(idx_t[:], conv_idx[t * P:(t + 1) * P, :])
            g2t = sbg.tile([P, K * R2], BF16, tag="g2")
            for k in range(K):
                nc.gpsimd.indirect_dma_start(
                    out=g2t[:, k * R2:(k + 1) * R2], out_offset=None, in_=h_full[:],
                    in_offset=bass.IndirectOffsetOnAxis(ap=idx_t[:, k:k + 1], axis=0),
                )
            t2 = sbg.tile([P, K * R2], BF16, tag="t2")
            nc.vector.transpose(
                out=t2[:].rearrange("p (k g v) -> p k g v", k=K, g=n_ob),
                in_=g2t[:].rearrange("p (k g c) -> p k g c", k=K, g=n_ob),
            )
            psums = [psp.tile([C2, BCOL], F32, space="PSUM",
                              name=f"ps2_{t}_{vb}", tag=f"psA_{vb}")
                     for vb in range(NVB)]
            for k in range(K):
                for ob in range(n_ob):
                    for vb in range(NVB):
                        nc.tensor.matmul(
                            out=psums[vb][:, :],
                            lhsT=w2_sb[vb * 32:(vb + 1) * 32,
                                       (k * n_ob + ob) * C2:(k * n_ob + ob + 1) * C2],
                            rhs=t2[vb * 32:(vb + 1) * 32,
                                   (k * n_ob + ob) * BCOL:(k * n_ob + ob + 1) * BCOL],
                            start=(k == 0 and ob == 0),
                            stop=(k == K - 1 and ob == n_ob - 1),
                            tile_position=(vb * 32, 0),
                        )
            tt = t % GRP
            if tt == 0:
                stage = sbg.tile([C2, B * GRP * 128], F32, tag="stage", bufs=2)
            stage_v = stage[:].rearrange("p (b g) -> p b g", b=B)
            for vb in range(NVB):
                sc = t * NVB + vb
                dump = sbg.tile([C2, BCOL], F32, tag="dump", bufs=2)
                nc.scalar.activation(
                    out=dump[:], in_=psums[vb][:], func=AF.Identity,
                    accum_out=stats2[:, 2 * sc:2 * sc + 1])
                dump2 = sbg.tile([C2, BCOL], F32, tag="dump", bufs=2)
                nc.scalar.activation(
                    out=dump2[:], in_=psums[vb][:], func=AF.Square,
                    accum_out=stats2[:, 2 * sc + 1:2 * sc + 2])
                nc.scalar.activation(
                    out=stage_v[:, :, tt * 128 + vb * 32: tt * 128 + (vb + 1) * 32],
                    in_=psums[vb][:].rearrange("p (b v) -> p b v", b=B),
                    func=AF.Copy)
            if tt == GRP - 1 or t == TPC - 1:
                g0 = (t // GRP) * GRP * 128
                gw = (tt + 1) * 128
                for b in range(B):
                    nc.sync.dma_start(
                        out[b, :, g0:g0 + gw],
                        stage[:, b * (GRP * 128): b * (GRP * 128) + gw],
                    )

        # ---- BN2 stats + fixup ----
        _stats_reduce(nc, sb1, stats2, ar_in2, ar_out2, NC, "st2")
        glob2 = sb1.tile([C2, 2], F32)
        nc.sync.dma_start(glob2[:], ar_out2[:])
        a2, c2 = _bn_coeffs(nc, sb1, glob2, g_sb[:, 2:3], g_sb[:, 3:4], N_stat, "bn2")
        # replicate coeffs to 128 partitions (2 batches per fixup tile)
        a2r = sb1.tile([128, 1], F32, name="a2r")
        c2r = sb1.tile([128, 1], F32, name="c2r")
        nc.vector.tensor_copy(a2r[0:C2, :], a2[:])
        nc.vector.tensor_copy(a2r[C2:2 * C2, :], a2[:])
        nc.vector.tensor_copy(c2r[0:C2, :], c2[:])
        nc.vector.tensor_copy(c2r[C2:2 * C2, :], c2[:])
        CH2 = 1024
        for b in range(0, B, 2):
            for v0 in range(0, Vs, CH2):
                vw = min(CH2, Vs - v0)
                fx = sbg.tile([128, CH2], F32, tag="fix", bufs=3)
                nc.sync.dma_start(fx[0:C2, :vw], out[b, :, v0:v0 + vw])
                nc.sync.dma_start(fx[C2:2 * C2, :vw], out[b + 1, :, v0:v0 + vw])
                if cfg.use_lrelu:
                    nc.scalar.activation(out=fx[:, :vw], in_=fx[:, :vw],
                                         func=AF.Prelu, bias=c2r[:], scale=a2r[:],
                                         alpha=cfg._alpha_sb[:, :])
                else:
                    nc.scalar.activation(out=fx[:, :vw], in_=fx[:, :vw],
                                         func=AF.Identity, bias=c2r[:], scale=a2r[:])
                    _lrelu(nc, cfg, sbg, fx[:, :vw])
                nc.sync.dma_start(out[b, :, v0:v0 + vw], fx[0:C2, :vw])
                nc.sync.dma_start(out[b + 1, :, v0:v0 + vw], fx[C2:2 * C2, :vw])


def _lrelu(nc, cfg, sbg, ap_):
    """In-place LeakyReLU on an SBUF AP."""
    if cfg.use_lrelu:
        nc.scalar.activation(out=ap_, in_=ap_, func=AF.Prelu,
                             alpha=cfg._alpha_sb[0:ap_.shape[0], :])
    else:
        tmp = sbg.tile([ap_.shape[0], ap_.free_size()], ap_.dtype,
                       tag="lrtmp", bufs=2)
        nc.vector.tensor_scalar_mul(out=tmp[:, :ap_.free_size()], in0=ap_,
                                     scalar1=ALPHA)
        nc.vector.tensor_tensor(out=ap_, in0=ap_,
                                in1=tmp[:, :ap_.free_size()], op=OP.max)


def _stats_reduce(nc, sb1, stats, ar_in, ar_out, NC, nm):
    C2 = stats.shape[0]
    red = sb1.tile([C2, 2], F32, name=f"{nm}_red")
    sv = stats[:].rearrange("p (s two) -> p two s", two=2)
    nc.vector.tensor_reduce(out=red[:, 0:1], in_=sv[:, 0:1, :],
                            axis=mybir.AxisListType.X, op=OP.add)
    nc.vector.tensor_reduce(out=red[:, 1:2], in_=sv[:, 1:2, :],
                            axis=mybir.AxisListType.X, op=OP.add)
    nc.gpsimd.dma_start(ar_in[:], red[:])
    nc.gpsimd.collective_compute(
        "AllReduce", OP.add, replica_groups=[list(range(NC))],
        ins=[ar_in[:].opt()], outs=[ar_out[:].opt()],
    )


def _bn_coeffs(nc, sb1, glob, gamma, beta, N, nm):
    C2 = glob.shape[0]
    tmp = sb1.tile([C2, 4], F32, name=f"{nm}_tmp")
    nc.scalar.activation(out=tmp[:, 0:1], in_=glob[:, 0:1], func=AF.Copy,
                         scale=1.0 / N)
    nc.scalar.activation(out=tmp[:, 1:2], in_=glob[:, 1:2], func=AF.Copy,
                         scale=1.0 / N)
    msq = sb1.tile([C2, 1], F32, name=f"{nm}_msq")
    nc.vector.tensor_tensor(out=msq[:], in0=tmp[:, 0:1], in1=tmp[:, 0:1],
                            op=OP.mult)
    nc.vector.tensor_tensor(out=tmp[:, 2:3], in0=tmp[:, 1:2], in1=msq[:],
                            op=OP.subtract)
    std = sb1.tile([C2, 1], F32, name=f"{nm}_std")
    epsb = sb1.tile([C2, 1], F32, name=f"{nm}_eps")
    nc.vector.memset(epsb[:], EPS)
    nc.scalar.activation(out=std[:], in_=tmp[:, 2:3], func=AF.Sqrt, bias=epsb[:])
    rstd = sb1.tile([C2, 1], F32, name=f"{nm}_rstd")
    nc.vector.reciprocal(rstd[:], std[:])
    a = sb1.tile([C2, 1], F32, name=f"{nm}_a")
    nc.vector.tensor_tensor(out=a[:], in0=gamma, in1=rstd[:], op=OP.mult)
    c = sb1.tile([C2, 1], F32, name=f"{nm}_c")
    am = sb1.tile([C2, 1], F32, name=f"{nm}_am")
    nc.vector.tensor_tensor(out=am[:], in0=a[:], in1=tmp[:, 0:1], op=OP.mult)
    nc.vector.tensor_tensor(out=c[:], in0=beta, in1=am[:], op=OP.subtract)
    return a, c


def _bn_rowmaps(nc, sb1, psp, ident, a, c, C2, B, n_ob, nm):
    """Build (1, R2) maps m[0, ob*B*32 + b*32 + om] = vec[ob*32 + om]."""
    aT_ps = psp.tile([1, C2], F32, space="PSUM", name=f"{nm}_aT", tag="psA_0")
    nc.tensor.transpose(out=aT_ps[:], in_=a[:], identity=ident[0:C2, 0:C2])
    aT = sb1.tile([1, C2], F32, name=f"{nm}_aTs")
    nc.vector.tensor_copy(aT[:], aT_ps[:])
    cT_ps = psp.tile([1, C2], F32, space="PSUM", name=f"{nm}_cT", tag="psA_1")
    nc.tensor.transpose(out=cT_ps[:], in_=c[:], identity=ident[0:C2, 0:C2])
    cT = sb1.tile([1, C2], F32, name=f"{nm}_cTs")
    nc.vector.tensor_copy(cT[:], cT_ps[:])
    R2 = B * C2
    am1 = sb1.tile([1, R2], F32, name=f"{nm}_amap1")
    cm1 = sb1.tile([1, R2], F32, name=f"{nm}_cmap1")
    for ob in range(n_ob):
        dst_a = am1[0:1, ob * B * 32:(ob + 1) * B * 32].rearrange(
            "p (b e) -> p b e", e=32)
        dst_c = cm1[0:1, ob * B * 32:(ob + 1) * B * 32].rearrange(
            "p (b e) -> p b e", e=32)
        src_a = bcast(aT[0:1, ob * 32:(ob + 1) * 32], 0, B)
        src_c = bcast(cT[0:1, ob * 32:(ob + 1) * 32], 0, B)
        nc.vector.tensor_copy(dst_a, src_a)
        nc.vector.tensor_copy(dst_c, src_c)
    am = sb1.tile([128, R2], F32, name=f"{nm}_amap")
    cm = sb1.tile([128, R2], F32, name=f"{nm}_cmap")
    nc.gpsimd.partition_broadcast(am[:], am1[:])
    nc.gpsimd.partition_broadcast(cm[:], cm1[:])
    return am, cm


def _bn_maps(nc, sb1, psp, ident, a, c, C2, nm):
    ac = sb1.tile([C2, 2], F32, name=f"{nm}_ac")
    nc.vector.tensor_copy(ac[:, 0:1], a[:])
    nc.vector.tensor_copy(ac[:, 1:2], c[:])
    acT_ps = psp.tile([2, C2], F32, space="PSUM", name=f"{nm}_acT", tag="psA_0")
    nc.tensor.transpose(out=acT_ps[:], in_=ac[:], identity=ident[0:C2, 0:C2])
    acT = sb1.tile([2, C2], F32, name=f"{nm}_acTs")
    nc.vector.tensor_copy(acT[:], acT_ps[:])
    am = sb1.tile([128, 32], F32, name=f"{nm}_amap")
    cm = sb1.tile([128, 32], F32, name=f"{nm}_cmap")
    for q in range(4):
        ob = q % 2 if C2 == 64 else 0
        nc.vector.tensor_copy(
            am[q * 32:(q + 1) * 32, :],
            acT[0:1, ob * 32:ob * 32 + 32].to_broadcast([32, 32]))
        nc.vector.tensor_copy(
            cm[q * 32:(q + 1) * 32, :],
            acT[1:2, ob * 32:ob * 32 + 32].to_broadcast([32, 32]))
    return am, cm


# ============================================================================
# Self-contained entry point: kernel(**inputs) -> full output
# ============================================================================
import concourse.bass_utils as bass_utils

_CACHE = {}


def kernel(x, w1, b1, g1, beta1, w2, b2, g2, beta2,
           conv_neigh_indices, down_neigh_indices):
    """DownBlock: IcoPool(mean) -> (conv-BN-LReLU) x2 on 8 trn2 NeuronCores.

    Takes full (unsharded) inputs, returns the full (B, 64, V_coarse) output.
    Conv biases b1/b2 are mathematically cancelled by the following BatchNorm
    (batch-stats mode), so they are unused.
    """
    x = np.asarray(x, np.float32)
    B, C1, VF = x.shape
    VC, K = np.asarray(conv_neigh_indices).shape
    C2 = np.asarray(w1).shape[0]
    cfg = Cfg(B=B, C1=C1, C2=C2, K=K, VF=VF, VC=VC, n_cores=8, use_lrelu=True)

    xt_rows, per_core = host_prep(
        cfg, x, w1, w2, g1, beta1, g2, beta2,
        conv_neigh_indices, down_neigh_indices)

    key = (B, C1, C2, K, VF, VC, xt_rows)
    # xt_rows depends on the index data; cache per full key
    if key not in _CACHE:
        _CACHE[key] = build(cfg, xt_rows)
    nc = _CACHE[key]

    res = bass_utils.run_bass_kernel_spmd(
        nc, per_core, core_ids=list(range(cfg.n_cores)))
    out = np.concatenate([r["out"] for r in res.results], axis=2)[:, :, :VC]
    return np.ascontiguousarray(out, dtype=np.float32)



# revision 2
# speedup vs baseline: 5.3591x; 5.3591x over previous
"""DownBlock kernel builder for trn2 (8-core SPMD, Bass/Tile).

See kernel.py for the pipeline description.
"""
import sys

sys.path.insert(0, "/opt/trn_rl_repo")

import contextlib

import ml_dtypes
import numpy as np

NP_BF16 = ml_dtypes.bfloat16

import concourse.bass as bass
import concourse.bacc as bacc
import concourse.mybir as mybir
import concourse.tile as tile
from concourse.masks import make_identity

F32 = mybir.dt.float32
BF16 = mybir.dt.bfloat16
I32 = mybir.dt.int32
AF = mybir.ActivationFunctionType
OP = mybir.AluOpType
EPS = 1e-5
ALPHA = 0.2
P = 128


def bcast(ap_obj, pos, count):
    """Insert a zero-stride dim of `count` at free-dim position `pos` (0 = first
    free dim)."""
    new = bass.AP(ap_obj.tensor, ap_obj.offset,
                  [list(d) for d in ap_obj.ap])
    new.ap.insert(pos + 1, [0, count])
    return new


class Cfg:
    def __init__(self, B=16, C1=32, C2=64, K=7, VF=163842, VC=40962, n_cores=8,
                 use_lrelu=True, stop_after=None):
        self.B, self.C1, self.C2, self.K = B, C1, C2, K
        self.VF, self.VC, self.n_cores = VF, VC, n_cores
        self.R1 = B * C1
        self.R2 = B * C2
        tiles_total = -(-VC // P)
        self.TPC = -(-tiles_total // n_cores)
        self.VC_pad = self.TPC * n_cores * P
        self.Vs = self.TPC * P
        self.use_lrelu = use_lrelu
        self.stop_after = stop_after
        assert C1 == 32 and C2 % 32 == 0


def host_prep(cfg, x, w1, w2, g1, beta1, g2, beta2, conv_neigh, down_neigh):
    B, C1, C2, K = cfg.B, cfg.C1, cfg.C2, cfg.K
    VF, VC = cfg.VF, cfg.VC
    x = np.asarray(x, np.float32)
    conv_neigh = np.asarray(conv_neigh).astype(np.int32)
    down_neigh = np.asarray(down_neigh).astype(np.int32)
    w1 = np.asarray(w1, np.float32)
    w2 = np.asarray(w2, np.float32)

    xT = np.ascontiguousarray(x.transpose(2, 0, 1).reshape(VF, B * C1))

    # W1rep[vb*32+c, k*C2+o] = w1[o, c*K+k] / K
    w1r = w1.reshape(C2, C1, K)             # [o, c, k]
    W1blk = (w1r.transpose(1, 2, 0) / K)    # [c, k, o]
    W1rep = np.tile(W1blk.reshape(C1, K * C2), (4, 1)).astype(NP_BF16)
    # W2rep[vb*32+om, (k*n_ob+ob)*C2+o2] = w2[o2, (ob*32+om)*K+k]
    n_ob = C2 // 32
    w2r = w2.reshape(C2, C2, K)             # [o2, o, k] with o = ob*32+om
    W2blk = w2r.reshape(C2, n_ob, 32, K).transpose(2, 3, 1, 0)  # [om, k, ob, o2]
    W2blk = W2blk.transpose(0, 1, 2, 3).reshape(32, K * n_ob * C2)
    W2rep = np.tile(W2blk, (4, 1)).astype(NP_BF16)

    gvec = np.stack([np.asarray(g1, np.float32), np.asarray(beta1, np.float32),
                     np.asarray(g2, np.float32), np.asarray(beta2, np.float32)],
                    axis=1)

    per_core = []
    for ci in range(cfg.n_cores):
        sh0 = ci * cfg.Vs
        vid = sh0 + np.arange(cfg.Vs)
        valid = vid < VC
        vv = np.where(valid, vid, 0)
        pool_idx = down_neigh[vv].copy()
        pool_idx[~valid] = 0
        ci_raw = conv_neigh[vv]
        conv_idx = (ci_raw // cfg.Vs) * (cfg.Vs + 1) + ci_raw % cfg.Vs
        conv_idx = conv_idx.astype(np.int32)
        conv_idx[~valid] = cfg.Vs  # core-0 zero row
        TPC = cfg.TPC
        TH0 = (TPC + 1) // 2
        nidx = K * P
        xt_halves, packed_halves = [], []
        for h, (t0, t1) in enumerate([(0, TH0), (TH0, TPC)]):
            sl = pool_idx[t0 * P:t1 * P, :]
            uniq, inv = np.unique(sl, return_inverse=True)
            assert len(uniq) < 32700, f"pool half rows {len(uniq)} > int16"
            xt_halves.append(xT[uniq])
            pidx = inv.reshape(sl.shape).astype(np.int16)
            nt = t1 - t0
            packed = np.zeros((nt * P, nidx // 16), np.int16)
            for t in range(nt):
                lst = pidx[t * P:(t + 1) * P, :].T.reshape(nidx)
                blk = lst.reshape(nidx // 16, 16).T
                packed[t * P:(t + 1) * P, :] = np.tile(blk, (8, 1))
            packed_halves.append(packed)
        per_core.append(dict(
            xt0=xt_halves[0], xt1=xt_halves[1],
            pool_idx=np.concatenate(packed_halves, axis=0),
            conv_idx=conv_idx,
            w1rep=W1rep, w2rep=W2rep, gvec=gvec,
        ))
    mx = max(max(pc["xt0"].shape[0], pc["xt1"].shape[0]) for pc in per_core)
    for pc in per_core:
        for nm in ("xt0", "xt1"):
            n = pc[nm].shape[0]
            if n < mx:
                pc[nm] = np.concatenate(
                    [pc[nm], np.zeros((mx - n, B * C1), np.float32)], 0)
            pc[nm] = np.ascontiguousarray(pc[nm]).astype(NP_BF16)
    return mx, per_core


def build(cfg, xt_rows_max):
    B, C2, K = cfg.B, cfg.C2, cfg.K
    R1 = cfg.R1
    n_ob = C2 // 32
    nc = bacc.Bacc("TRN2", target_bir_lowering=False, debug=False,
                   num_devices=cfg.n_cores)
    xtt = (nc.dram_tensor("xt0", [xt_rows_max, R1], BF16, kind="ExternalInput").ap(),
           nc.dram_tensor("xt1", [xt_rows_max, R1], BF16, kind="ExternalInput").ap())
    pool_idx = nc.dram_tensor("pool_idx", [cfg.Vs, K * P // 16],
                          mybir.dt.int16, kind="ExternalInput").ap()
    conv_idx = nc.dram_tensor("conv_idx", [cfg.Vs, K], I32, kind="ExternalInput").ap()
    w1rep = nc.dram_tensor("w1rep", [128, K * C2], BF16, kind="ExternalInput").ap()
    w2rep = nc.dram_tensor("w2rep", [128, K * n_ob * C2], BF16,
                           kind="ExternalInput").ap()
    gvec = nc.dram_tensor("gvec", [C2, 4], F32, kind="ExternalInput").ap()
    out = nc.dram_tensor("out", [B, C2, cfg.Vs], F32, kind="ExternalOutput").ap()

    with tile.TileContext(nc) as tc:
        build_body(tc, cfg, xtt, pool_idx, conv_idx, w1rep, w2rep, gvec, out)
    nc.compile()
    return nc


def build_body(tc, cfg, xt, pool_idx, conv_idx, w1rep, w2rep, gvec, out):
    nc = tc.nc
    B, C1, C2, K = cfg.B, cfg.C1, cfg.C2, cfg.K
    R1, R2 = cfg.R1, cfg.R2
    TPC, Vs, VC_pad = cfg.TPC, cfg.Vs, cfg.VC_pad
    NC = cfg.n_cores
    n_ob = C2 // 32
    N_stat = float(B * cfg.VC)
    NVB = 4
    BCOL = B * 32   # psum free size per tile (b, vr)

    ctx = contextlib.ExitStack()
    with ctx:
        dram = ctx.enter_context(tc.tile_pool(name="dram", bufs=1, space="DRAM"))
        sb1 = ctx.enter_context(tc.tile_pool(name="sb1", bufs=1))
        sbg = ctx.enter_context(tc.tile_pool(name="sbg", bufs=2))
        psp = ctx.enter_context(tc.tile_pool(name="psp", bufs=2, space="PSUM"))

        xp_shard = dram.tile([Vs + 1, R1], BF16, name="xp_shard")
        xp_full = dram.tile([(Vs + 1) * NC, R1], BF16, name="xp_full",
                            addr_space="Shared")
        h_shard = dram.tile([Vs + 1, R2], BF16, name="h_shard")
        h_full = dram.tile([(Vs + 1) * NC, R2], BF16, name="h_full",
                           addr_space="Shared")
        ar_in = dram.tile([C2, 2], F32, name="ar_in")
        ar_out = dram.tile([C2, 2], F32, name="ar_out", addr_space="Shared")
        ar_in2 = dram.tile([C2, 2], F32, name="ar_in2")
        ar_out2 = dram.tile([C2, 2], F32, name="ar_out2", addr_space="Shared")

        w1_sb = sb1.tile([128, K * C2], BF16)
        nc.sync.dma_start(w1_sb[:], w1rep[:])
        w2_sb = sb1.tile([128, K * n_ob * C2], BF16)
        nc.sync.dma_start(w2_sb[:], w2rep[:])
        g_sb = sb1.tile([C2, 4], F32)
        nc.sync.dma_start(g_sb[:], gvec[:])
        ident = sb1.tile([128, 128], F32)
        make_identity(nc, ident[:])
        zero_bf = sb1.tile([128, R2], BF16)
        nc.vector.memset(zero_bf[:], 0.0)
        zero_sb = zero_bf
        alpha_sb = sb1.tile([128, 1], F32)
        nc.vector.memset(alpha_sb[:], ALPHA)
        cfg._alpha_sb = alpha_sb

        # h_pre lives in SBUF: [128 = (vb, vv) tile-local row, TPC * R2] bf16
        h_pre_sb = sb1.tile([128, TPC * R2], BF16, name="h_pre_sb")
        hp_t = h_pre_sb[:].rearrange("p (t c) -> p t c", t=TPC)
        stats1 = sb1.tile([C2, TPC * NVB * 2], F32)
        stats2 = sb1.tile([C2, TPC * NVB * 2], F32)

        # =========== P1: pool ===========
        NIDX = K * P
        TH0 = (TPC + 1) // 2
        for t in range(TPC):
            idx_t = sbg.tile([P, NIDX // 16], mybir.dt.int16, tag="pidx", bufs=3)
            nc.sync.dma_start(idx_t[:], pool_idx[t * P:(t + 1) * P, :])
            gp = sbg.tile([P, K * R1], BF16, tag="g1")
            nc.gpsimd.dma_gather(
                out_ap=gp[:].rearrange("p (n e) -> p n e", e=R1),
                in_ap=xt[0][:] if t < TH0 else xt[1][:], idxs_ap=idx_t[:],
                num_idxs=NIDX, num_idxs_reg=NIDX, elem_size=R1,
            )
            acc = sbg.tile([P, R1], BF16, tag="poolacc", bufs=2)
            with nc.allow_low_precision(reason="pool mean of 7 in bf16"):
                nc.vector.tensor_reduce(
                    out=acc[:],
                    in_=bass.AP(gp.tensor, gp[:].offset,
                                [list(gp[:].ap[0]), [1, R1], [R1, K]]),
                    axis=mybir.AxisListType.X, op=OP.add)
            nc.sync.dma_start(xp_shard[t * P:(t + 1) * P, :], acc[:])

        nc.sync.dma_start(xp_shard[Vs:Vs + 1, :], zero_sb[0:1, 0:R1])
        if cfg.stop_after == "pool":
            fin = sbg.tile([P, R1], F32, tag="fin")
            nc.sync.dma_start(fin[:], xp_shard[0:P, :])
            nc.sync.dma_start(out[0, 0:64, 0:R1], fin[0:64, :])
            return
        nc.gpsimd.collective_compute(
            "AllGather", OP.bypass, replica_groups=[list(range(NC))],
            ins=[xp_shard[:].opt()], outs=[xp_full[:].opt()],
        )

        # =========== C1: conv1 ===========
        for t in range(TPC):
            idx_t = sbg.tile([P, K], I32, tag="cidx", bufs=3)
            nc.sync.dma_start(idx_t[:], conv_idx[t * P:(t + 1) * P, :])
            g1t = sbg.tile([P, K * R1], BF16, tag="g1")
            for k in range(K):
                nc.gpsimd.indirect_dma_start(
                    out=g1t[:, k * R1:(k + 1) * R1], out_offset=None, in_=xp_full[:],
                    in_offset=bass.IndirectOffsetOnAxis(ap=idx_t[:, k:k + 1], axis=0),
                )
            t1 = sbg.tile([P, K * R1], BF16, tag="t1")
            nc.vector.transpose(
                out=t1[:].rearrange("p (k b v) -> p k b v", k=K, b=B),
                in_=g1t[:].rearrange("p (k b c) -> p k b c", k=K, b=B),
            )
            psums = [psp.tile([C2, BCOL], F32, space="PSUM",
                              name=f"ps1_{t}_{vb}", tag=f"psA_{vb}")
                     for vb in range(NVB)]
            for k in range(K):
                for vb in range(NVB):
                    nc.tensor.matmul(
                        out=psums[vb][:, :],
                        lhsT=w1_sb[vb * 32:(vb + 1) * 32, k * C2:(k + 1) * C2],
                        rhs=t1[vb * 32:(vb + 1) * 32, k * R1:(k + 1) * R1],
                        start=(k == 0), stop=(k == K - 1),
                        tile_position=(vb * 32, 0),
                    )
            for vb in range(NVB):
                sc = t * NVB + vb
                dump = sbg.tile([C2, BCOL], BF16, tag="dump", bufs=2)
                nc.scalar.activation(
                    out=dump[:], in_=psums[vb][:], func=AF.Identity,
                    accum_out=stats1[:, 2 * sc:2 * sc + 1])
                dump2 = sbg.tile([C2, BCOL], BF16, tag="dump", bufs=2)
                nc.scalar.activation(
                    out=dump2[:], in_=psums[vb][:], func=AF.Square,
                    accum_out=stats1[:, 2 * sc + 1:2 * sc + 2])
                ht = sbg.tile([C2, BCOL], BF16, tag="ht", bufs=2)
                nc.vector.transpose(
                    out=ht[:].rearrange("p (b v) -> p b v", b=B),
                    in_=dump[:].rearrange("p (b v) -> p b v", b=B),
                )
                for ob in range(n_ob):
                    eng = nc.sync if vb % 2 == 0 else nc.scalar
                    eng.dma_start(
                        hp_t[vb * 32:(vb + 1) * 32, t,
                             ob * BCOL:(ob + 1) * BCOL],
                        ht[ob * 32:(ob + 1) * 32, :])

        # ---- BN1: stats AR, coeffs, apply, write h, AG ----
        _stats_reduce(nc, sb1, stats1, ar_in, ar_out, NC, "st1")
        glob1 = sb1.tile([C2, 2], F32)
        nc.sync.dma_start(glob1[:], ar_out[:])
        a1, c1 = _bn_coeffs(nc, sb1, glob1, g_sb[:, 0:1], g_sb[:, 1:2], N_stat, "bn1")
        a1m, c1m = _bn_rowmaps(nc, sb1, psp, ident, a1, c1, C2, B, n_ob, "b1m")

        # BN1 apply in-place on SBUF h_pre, then stream to h_shard
        nc.vector.tensor_tensor(out=hp_t, in0=hp_t,
                                in1=bcast(a1m[:, :], 0, TPC), op=OP.mult)
        nc.vector.tensor_tensor(out=hp_t, in0=hp_t,
                                in1=bcast(c1m[:, :], 0, TPC), op=OP.add)
        _lrelu(nc, cfg, sbg, h_pre_sb[:])
        for t in range(TPC):
            nc.sync.dma_start(h_shard[t * P:(t + 1) * P, :], hp_t[:, t, :])
        nc.sync.dma_start(h_shard[Vs:Vs + 1, :], zero_bf[0:1, :])
        if cfg.stop_after == "conv1":
            fin = sbg.tile([P, R2], BF16, tag="finb")
            nc.sync.dma_start(fin[:], h_shard[0:P, :])
            fin2 = sbg.tile([P, R2], F32, tag="finb2")
            nc.vector.tensor_copy(fin2[:], fin[:])
            nc.sync.dma_start(out[0, 0:64, 0:R2], fin2[0:64, :])
            return
        nc.gpsimd.collective_compute(
            "AllGather", OP.bypass, replica_groups=[list(range(NC))],
            ins=[h_shard[:].opt()], outs=[h_full[:].opt()],
        )

        # =========== C2: conv2 ===========
        GRP = 4
        stage = None
        for t in range(TPC):
            idx_t = sbg.tile([P, K], I32, tag="cidx2", bufs=3)
            nc.sync.dma_start(idx_t[:], conv_idx[t * P:(t + 1) * P, :])
            g2t = sbg.tile([P, K * R2], BF16, tag="g1")
            for k in range(K):
                nc.gpsimd.indirect_dma_start(
                    out=g2t[:, k * R2:(k + 1) * R2], out_offset=None, in_=h_full[:],
                    in_offset=bass.IndirectOffsetOnAxis(ap=idx_t[:, k:k + 1], axis=0),
                )
            t2 = sbg.tile([P, K * R2], BF16, tag="t1")
            nc.vector.transpose(
                out=t2[:].rearrange("p (k g v) -> p k g v", k=K, g=n_ob),
                in_=g2t[:].rearrange("p (k g c) -> p k g c", k=K, g=n_ob),
            )
            psums = [psp.tile([C2, BCOL], F32, space="PSUM",
                              name=f"ps2_{t}_{vb}", tag=f"psA_{vb}")
                     for vb in range(NVB)]
            for k in range(K):
                for ob in range(n_ob):
                    for vb in range(NVB):
                        nc.tensor.matmul(
                            out=psums[vb][:, :],
                            lhsT=w2_sb[vb * 32:(vb + 1) * 32,
                                       (k * n_ob + ob) * C2:(k * n_ob + ob + 1) * C2],
                            rhs=t2[vb * 32:(vb + 1) * 32,
                                   (k * n_ob + ob) * BCOL:(k * n_ob + ob + 1) * BCOL],
                            start=(k == 0 and ob == 0),
                            stop=(k == K - 1 and ob == n_ob - 1),
                            tile_position=(vb * 32, 0),
                        )
            tt = t % GRP
            if tt == 0:
                stage = sbg.tile([C2, B * GRP * 128], F32, tag="stage", bufs=1)
            stage_v = stage[:].rearrange("p (b g) -> p b g", b=B)
            for vb in range(NVB):
                sc = t * NVB + vb
                dump = sbg.tile([C2, BCOL], BF16, tag="dump", bufs=2)
                nc.scalar.activation(
                    out=dump[:], in_=psums[vb][:], func=AF.Identity,
                    accum_out=stats2[:, 2 * sc:2 * sc + 1])
                dump2 = sbg.tile([C2, BCOL], BF16, tag="dump", bufs=2)
                nc.scalar.activation(
                    out=dump2[:], in_=psums[vb][:], func=AF.Square,
                    accum_out=stats2[:, 2 * sc + 1:2 * sc + 2])
                nc.scalar.activation(
                    out=stage_v[:, :, tt * 128 + vb * 32: tt * 128 + (vb + 1) * 32],
                    in_=psums[vb][:].rearrange("p (b v) -> p b v", b=B),
                    func=AF.Copy)
            if tt == GRP - 1 or t == TPC - 1:
                g0 = (t // GRP) * GRP * 128
                gw = (tt + 1) * 128
                for b in range(B):
                    nc.sync.dma_start(
                        out[b, :, g0:g0 + gw],
                        stage[:, b * (GRP * 128): b * (GRP * 128) + gw],
                    )

        # ---- BN2 stats + fixup ----
        _stats_reduce(nc, sb1, stats2, ar_in2, ar_out2, NC, "st2")
        glob2 = sb1.tile([C2, 2], F32)
        nc.sync.dma_start(glob2[:], ar_out2[:])
        a2, c2 = _bn_coeffs(nc, sb1, glob2, g_sb[:, 2:3], g_sb[:, 3:4], N_stat, "bn2")
        # replicate coeffs to 128 partitions (2 batches per fixup tile)
        a2r = sb1.tile([128, 1], F32, name="a2r")
        c2r = sb1.tile([128, 1], F32, name="c2r")
        nc.vector.tensor_copy(a2r[0:C2, :], a2[:])
        nc.vector.tensor_copy(a2r[C2:2 * C2, :], a2[:])
        nc.vector.tensor_copy(c2r[0:C2, :], c2[:])
        nc.vector.tensor_copy(c2r[C2:2 * C2, :], c2[:])
        CH2 = 1024
        for b in range(0, B, 2):
            for v0 in range(0, Vs, CH2):
                vw = min(CH2, Vs - v0)
                fx = sbg.tile([128, CH2], F32, tag="fix", bufs=1)
                nc.sync.dma_start(fx[0:C2, :vw], out[b, :, v0:v0 + vw])
                nc.sync.dma_start(fx[C2:2 * C2, :vw], out[b + 1, :, v0:v0 + vw])
                if cfg.use_lrelu:
                    nc.scalar.activation(out=fx[:, :vw], in_=fx[:, :vw],
                                         func=AF.Prelu, bias=c2r[:], scale=a2r[:],
                                         alpha=cfg._alpha_sb[:, :])
                else:
                    nc.scalar.activation(out=fx[:, :vw], in_=fx[:, :vw],
                                         func=AF.Identity, bias=c2r[:], scale=a2r[:])
                    _lrelu(nc, cfg, sbg, fx[:, :vw])
                nc.sync.dma_start(out[b, :, v0:v0 + vw], fx[0:C2, :vw])
                nc.sync.dma_start(out[b + 1, :, v0:v0 + vw], fx[C2:2 * C2, :vw])


def _lrelu(nc, cfg, sbg, ap_):
    """In-place LeakyReLU on an SBUF AP."""
    if cfg.use_lrelu:
        nc.scalar.activation(out=ap_, in_=ap_, func=AF.Prelu,
                             alpha=cfg._alpha_sb[0:ap_.shape[0], :])
    else:
        tmp = sbg.tile([ap_.shape[0], ap_.free_size()], ap_.dtype,
                       tag="lrtmp", bufs=2)
        nc.vector.tensor_scalar_mul(out=tmp[:, :ap_.free_size()], in0=ap_,
                                     scalar1=ALPHA)
        nc.vector.tensor_tensor(out=ap_, in0=ap_,
                                in1=tmp[:, :ap_.free_size()], op=OP.max)


def _stats_reduce(nc, sb1, stats, ar_in, ar_out, NC, nm):
    C2 = stats.shape[0]
    red = sb1.tile([C2, 2], F32, name=f"{nm}_red")
    sv = stats[:].rearrange("p (s two) -> p two s", two=2)
    nc.vector.tensor_reduce(out=red[:, 0:1], in_=sv[:, 0:1, :],
                            axis=mybir.AxisListType.X, op=OP.add)
    nc.vector.tensor_reduce(out=red[:, 1:2], in_=sv[:, 1:2, :],
                            axis=mybir.AxisListType.X, op=OP.add)
    nc.gpsimd.dma_start(ar_in[:], red[:])
    nc.gpsimd.collective_compute(
        "AllReduce", OP.add, replica_groups=[list(range(NC))],
        ins=[ar_in[:].opt()], outs=[ar_out[:].opt()],
    )


def _bn_coeffs(nc, sb1, glob, gamma, beta, N, nm):
    C2 = glob.shape[0]
    tmp = sb1.tile([C2, 4], F32, name=f"{nm}_tmp")
    nc.scalar.activation(out=tmp[:, 0:1], in_=glob[:, 0:1], func=AF.Copy,
                         scale=1.0 / N)
    nc.scalar.activation(out=tmp[:, 1:2], in_=glob[:, 1:2], func=AF.Copy,
                         scale=1.0 / N)
    msq = sb1.tile([C2, 1], F32, name=f"{nm}_msq")
    nc.vector.tensor_tensor(out=msq[:], in0=tmp[:, 0:1], in1=tmp[:, 0:1],
                            op=OP.mult)
    nc.vector.tensor_tensor(out=tmp[:, 2:3], in0=tmp[:, 1:2], in1=msq[:],
                            op=OP.subtract)
    std = sb1.tile([C2, 1], F32, name=f"{nm}_std")
    epsb = sb1.tile([C2, 1], F32, name=f"{nm}_eps")
    nc.vector.memset(epsb[:], EPS)
    nc.scalar.activation(out=std[:], in_=tmp[:, 2:3], func=AF.Sqrt, bias=epsb[:])
    rstd = sb1.tile([C2, 1], F32, name=f"{nm}_rstd")
    nc.vector.reciprocal(rstd[:], std[:])
    a = sb1.tile([C2, 1], F32, name=f"{nm}_a")
    nc.vector.tensor_tensor(out=a[:], in0=gamma, in1=rstd[:], op=OP.mult)
    c = sb1.tile([C2, 1], F32, name=f"{nm}_c")
    am = sb1.tile([C2, 1], F32, name=f"{nm}_am")
    nc.vector.tensor_tensor(out=am[:], in0=a[:], in1=tmp[:, 0:1], op=OP.mult)
    nc.vector.tensor_tensor(out=c[:], in0=beta, in1=am[:], op=OP.subtract)
    return a, c


def _bn_rowmaps(nc, sb1, psp, ident, a, c, C2, B, n_ob, nm):
    """Build (1, R2) maps m[0, ob*B*32 + b*32 + om] = vec[ob*32 + om]."""
    aT_ps = psp.tile([1, C2], F32, space="PSUM", name=f"{nm}_aT", tag="psA_0")
    nc.tensor.transpose(out=aT_ps[:], in_=a[:], identity=ident[0:C2, 0:C2])
    aT = sb1.tile([1, C2], F32, name=f"{nm}_aTs")
    nc.vector.tensor_copy(aT[:], aT_ps[:])
    cT_ps = psp.tile([1, C2], F32, space="PSUM", name=f"{nm}_cT", tag="psA_1")
    nc.tensor.transpose(out=cT_ps[:], in_=c[:], identity=ident[0:C2, 0:C2])
    cT = sb1.tile([1, C2], F32, name=f"{nm}_cTs")
    nc.vector.tensor_copy(cT[:], cT_ps[:])
    R2 = B * C2
    am1 = sb1.tile([1, R2], F32, name=f"{nm}_amap1")
    cm1 = sb1.tile([1, R2], F32, name=f"{nm}_cmap1")
    for ob in range(n_ob):
        dst_a = am1[0:1, ob * B * 32:(ob + 1) * B * 32].rearrange(
            "p (b e) -> p b e", e=32)
        dst_c = cm1[0:1, ob * B * 32:(ob + 1) * B * 32].rearrange(
            "p (b e) -> p b e", e=32)
        src_a = bcast(aT[0:1, ob * 32:(ob + 1) * 32], 0, B)
        src_c = bcast(cT[0:1, ob * 32:(ob + 1) * 32], 0, B)
        nc.vector.tensor_copy(dst_a, src_a)
        nc.vector.tensor_copy(dst_c, src_c)
    am = sb1.tile([128, R2], mybir.dt.bfloat16, name=f"{nm}_amap")
    cm = sb1.tile([128, R2], mybir.dt.bfloat16, name=f"{nm}_cmap")
    am32 = sb1.tile([1, R2], mybir.dt.bfloat16, name=f"{nm}_am32")
    cm32 = sb1.tile([1, R2], mybir.dt.bfloat16, name=f"{nm}_cm32")
    nc.vector.tensor_copy(am32[:], am1[:])
    nc.vector.tensor_copy(cm32[:], cm1[:])
    nc.gpsimd.partition_broadcast(am[:], am32[:])
    nc.gpsimd.partition_broadcast(cm[:], cm32[:])
    return am, cm


def _bn_maps(nc, sb1, psp, ident, a, c, C2, nm):
    ac = sb1.tile([C2, 2], F32, name=f"{nm}_ac")
    nc.vector.tensor_copy(ac[:, 0:1], a[:])
    nc.vector.tensor_copy(ac[:, 1:2], c[:])
    acT_ps = psp.tile([2, C2], F32, space="PSUM", name=f"{nm}_acT", tag="psA_0")
    nc.tensor.transpose(out=acT_ps[:], in_=ac[:], identity=ident[0:C2, 0:C2])
    acT = sb1.tile([2, C2], F32, name=f"{nm}_acTs")
    nc.vector.tensor_copy(acT[:], acT_ps[:])
    am = sb1.tile([128, 32], F32, name=f"{nm}_amap")
    cm = sb1.tile([128, 32], F32, name=f"{nm}_cmap")
    for q in range(4):
        ob = q % 2 if C2 == 64 else 0
        nc.vector.tensor_copy(
            am[q * 32:(q + 1) * 32, :],
            acT[0:1, ob * 32:ob * 32 + 32].to_broadcast([32, 32]))
        nc.vector.tensor_copy(
            cm[q * 32:(q + 1) * 32, :],
            acT[1:2, ob * 32:ob * 32 + 32].to_broadcast([32, 32]))
    return am, cm


# ============================================================================
# Self-contained entry point: kernel(**inputs) -> full output
# ============================================================================
import concourse.bass_utils as bass_utils

_CACHE = {}


def kernel(x, w1, b1, g1, beta1, w2, b2, g2, beta2,
           conv_neigh_indices, down_neigh_indices):
    """DownBlock: IcoPool(mean) -> (conv-BN-LReLU) x2 on 8 trn2 NeuronCores.

    Takes full (unsharded) inputs, returns the full (B, 64, V_coarse) output.
    Conv biases b1/b2 are mathematically cancelled by the following BatchNorm
    (batch-stats mode), so they are unused.
    """
    x = np.asarray(x, np.float32)
    B, C1, VF = x.shape
    VC, K = np.asarray(conv_neigh_indices).shape
    C2 = np.asarray(w1).shape[0]
    cfg = Cfg(B=B, C1=C1, C2=C2, K=K, VF=VF, VC=VC, n_cores=8, use_lrelu=True)

    xt_rows, per_core = host_prep(
        cfg, x, w1, w2, g1, beta1, g2, beta2,
        conv_neigh_indices, down_neigh_indices)

    key = (B, C1, C2, K, VF, VC, xt_rows)
    # xt_rows depends on the index data; cache per full key
    if key not in _CACHE:
        _CACHE[key] = build(cfg, xt_rows)
    nc = _CACHE[key]

    res = bass_utils.run_bass_kernel_spmd(
        nc, per_core, core_ids=list(range(cfg.n_cores)))
    out = np.concatenate([r["out"] for r in res.results], axis=2)[:, :, :VC]
    return np.ascontiguousarray(out, dtype=np.float32)

